# revision 51
# baseline (speedup 1.0000x reference)
"""AttentionBlock (GroupNorm + single-head full attention + residual) on 8 trn2 cores.

Sharding: core i -> batch i//4, query strip (i%4)*1024 .. +1024. Each core
computes its batch's full K/V (duplicated across the 4 cores sharing the
batch). The host rotates each core's copy of x so its query strip sits at
token rows 0..1023 (group-norm statistics and attention key-sums are
permutation-invariant over tokens), letting one SPMD program serve all cores.

Differences vs the bf16 baseline (283.8us):
  - All heavy matmuls run fp8e4m3 with MatmulPerfMode.DoubleRow (virtual
    256-row contraction, ~1.8x streaming throughput): QKV projections,
    scores, exp-weights @ V, rowsum, and the output projection. Operands are
    stored channel-pair interleaved ([128, 2, N] tiles; element [p, j, n] is
    contraction row j*128+p).
  - x arrives channel-major fp8 (host pre-transpose), eliminating the PE
    transpose + copy pipeline of P1.
  - GroupNorm statistics come from DVE bn_stats/bn_aggr over the resident
    channel-major x (no PE stats matmuls, no Square pass); per-channel
    mean/var are PE-transposed to rows, pooled to 32 groups on the free dim,
    refined with Newton-Raphson rsqrt, and broadcast back to per-channel
    scale/bias via a small select-matrix matmul.
  - exp(S*scale - 2) is written directly as fp8e4m3 (logits for these
    normalized inputs are ~N(0,1.2), |S|<7, so the fixed shift keeps
    exp in [e^-9, e^5] - inside e4m3 range; the shift cancels in the
    softmax quotient). Row sums use the same fp8 values, so the softmax
    stays consistent. Attention output is normalized (rowsum reciprocal
    broadcast across partitions) before the fp8 output projection.
  - v/proj biases fold into the residual on host (xres + bv@wp + bp);
    k bias is dropped (softmax shift-invariant); q bias folds into the
    PSUM evacuation of Q^T.
HAM warm-up dummy matmuls run during the stats phase to hold the PE clock.
"""

import os
import numpy as np
from contextlib import ExitStack

import concourse.bass as bass
import concourse.bacc as bacc
import concourse.tile as tile
from concourse import mybir
from concourse.bass_utils import run_bass_kernel_spmd

B, H, W, C = 2, 64, 64, 512
T = H * W                 # 4096 tokens per batch
NCORES = 8
QS = 1024                 # queries per core
GROUPS, GSIZE = 32, 16
EPS = 1e-5
SCALE = float(C) ** -0.5
SHIFT = 2.0               # constant logit shift before exp (cancels in softmax)
F32 = mybir.dt.float32
F8 = mybir.dt.float8e4
DRM = mybir.MatmulPerfMode.DoubleRow
NCH = C // 128            # 4 channel chunks
NPAIR = 2                 # channel-chunk pairs (DoubleRow contraction groups)
NW = T // 512             # 8 token windows
NQW = QS // 512           # 2 query windows
NKT = T // 128            # 32 key subtiles
NBLK = QS // 512          # 2 attention q-blocks
NSUB = 4                  # 128-query subtiles per block


def _build():
    nc = bacc.Bacc(None, target_bir_lowering=False)

    xt_h = nc.declare_dram_parameter("xt", [NPAIR, 128, 2, T], F8, isOutput=False)
    xresb_h = nc.declare_dram_parameter("xresb", [QS, C], F32, isOutput=False)
    wkq_h = nc.declare_dram_parameter("wkq", [NPAIR, 128, 2, C], F8, isOutput=False)
    wv_h = nc.declare_dram_parameter("wv", [NPAIR, 128, 2, C], F8, isOutput=False)
    wp_h = nc.declare_dram_parameter("wp", [NPAIR, 128, 2, C], F8, isOutput=False)
    bq_h = nc.declare_dram_parameter("bq", [C], F32, isOutput=False)
    gamma_h = nc.declare_dram_parameter("gamma", [C], F32, isOutput=False)
    beta_h = nc.declare_dram_parameter("beta", [C], F32, isOutput=False)
    sel_h = nc.declare_dram_parameter("selmat", [32, 512], F32, isOutput=False)
    selp_h = nc.declare_dram_parameter("selpool", [128, NCH, 32], F32, isOutput=False)
    ones_h = nc.declare_dram_parameter("ones8", [128, 2, 16], F8, isOutput=False)
    out_h = nc.declare_dram_parameter("out", [QS, C], F32, isOutput=True)

    with tile.TileContext(nc) as tc, ExitStack() as ctx:
        persist = ctx.enter_context(tc.tile_pool(name="persist", bufs=1))
        small = ctx.enter_context(tc.tile_pool(name="small", bufs=1))

        bigpool = ctx.enter_context(tc.tile_pool(name="bigpool", bufs=1))
        # resident channel-major tensors, channel-pair interleaved
        xt_t = [bigpool.tile([128, 2, T], F8, tag=f"xt{p}", name=f"xt{p}")
                for p in range(NPAIR)]
        # normalized h, channel-major, one tile per (window, pair) so each
        # window's projections don't serialize against the next window's
        # normalize (K is never materialized - wq@wk^T is folded into one
        # matrix applied to the query side)
        hw_win = [[bigpool.tile([128, 2, 512], F8, tag=f"hw{w}_{p}",
                                name=f"hw{w}_{p}") for p in range(NPAIR)]
                  for w in range(NW)]
        qts_t = [bigpool.tile([128, 2, QS], F8, tag=f"qts{p}", name=f"qts{p}")
                 for p in range(NPAIR)]
        v_big = bigpool.tile([128, NKT, C], F8, tag="vbig", name="vbig")

        wpool = ctx.enter_context(tc.tile_pool(name="wpool", bufs=1))
        wkq_t = [wpool.tile([128, 2, C], F8, tag=f"wkq{p}", name=f"wkq{p}") for p in range(NPAIR)]
        wv_t = [wpool.tile([128, 2, C], F8, tag=f"wv{p}", name=f"wv{p}") for p in range(NPAIR)]
        wp_t = [persist.tile([128, 2, C], F8, tag=f"wp{p}", name=f"wp{p}") for p in range(NPAIR)]

        # x loads first, spread across the three DMA queues, so bn_stats
        # starts as early as possible
        xq = [nc.sync, nc.gpsimd, nc.scalar, nc.sync]
        for p in range(NPAIR):
            for j in range(2):
                xq[2 * p + j].dma_start(out=xt_t[p][:, j, :], in_=xt_h[p, :, j, :])

        ones8 = persist.tile([128, 2, 16], F8, tag="ones8", name="ones8")
        nc.scalar.dma_start(out=ones8, in_=ones_h[:, :, :])
        for p in range(NPAIR):
            nc.scalar.dma_start(out=wkq_t[p], in_=wkq_h[p])
            nc.scalar.dma_start(out=wv_t[p], in_=wv_h[p])
            nc.scalar.dma_start(out=wp_t[p], in_=wp_h[p])

        # per-channel vectors as [128, NCH] (column cc = channel chunk cc)
        def vec_tile(h, name):
            t = small.tile([128, NCH], F32, tag=name)
            nc.scalar.dma_start(out=t, in_=h.rearrange("(a p) -> p a", p=128))
            return t

        gamma_sb = vec_tile(gamma_h, "gamma")
        beta_sb = vec_tile(beta_h, "beta")
        bq_sb = vec_tile(bq_h, "bq")
        sel_sb = small.tile([32, 512], F32, tag="sel_sb", name="sel_sb")
        nc.scalar.dma_start(out=sel_sb, in_=sel_h[:, :])
        selp_sb = small.tile([128, NCH, 32], F32, tag="selp_sb", name="selp_sb")
        nc.scalar.dma_start(out=selp_sb, in_=selp_h[:, :, :])

        scale_all = small.tile([128, NCH, 1], F32, tag="scale_all", name="scale_all")
        bias_all = small.tile([128, NCH, 1], F32, tag="bias_all", name="bias_all")
        scale_t = [scale_all[:, c, :] for c in range(NCH)]
        bias_t = [bias_all[:, c, :] for c in range(NCH)]
        shift_t = small.tile([128, 1], F32, tag="shift_t", name="shift_t")
        nc.vector.memset(shift_t, -SHIFT)

        # PE warm-up / keep-alive: dummy matmuls hold the HAM clock at 2.4GHz
        warm_sb = small.tile([128, 512], F32, tag="warm_sb", name="warm_sb")
        nc.vector.memset(warm_sb, 0.0)

        # ================= P1: group-norm statistics (DVE bn_stats) ============
        # Everything stays on partitions: per-channel (mean, var, mean^2) rows
        # are pooled to the 32 groups with a tiny select matmul (contraction
        # over the partition/channel dim), so no slow 1-partition row ops.
        with tc.tile_pool(name="p1ps", bufs=1, space="PSUM") as p1ps, \
             tc.tile_pool(name="p1sb", bufs=1) as p1sb:

            def keepalive(n, dep=None):
                # dep (optional) delays the dummy matmuls until that tile is
                # ready, spreading them across the stats phase so the HAM
                # clock gate never sees a >3.4us PE-idle window
                for _ in range(n):
                    kps = p1ps.tile([128, 512], F32, tag="keep", name="keep", bufs=1)
                    lhs = dep if dep is not None else warm_sb[:, 0:128]
                    nc.tensor.matmul(kps[0:lhs.shape[-1], :], lhs,
                                     warm_sb[0:lhs.shape[0], :],
                                     start=True, stop=True)

            keepalive(40)
            rhs3 = []
            for cc in range(NCH):
                p, j = cc // 2, cc % 2
                bn6 = p1sb.tile([128, 8, 6], F32, tag=f"bn6_{cc}", name=f"bn6_{cc}")
                for s in range(8):
                    nc.vector.bn_stats(bn6[:, s, :], xt_t[p][:, j, s * 512:(s + 1) * 512])
                r3 = p1sb.tile([128, 3], F32, tag=f"bn2_{cc}", name=f"bn2_{cc}")
                nc.vector.bn_aggr(r3[:, 0:2], bn6.rearrange("p a (b c) -> p (a b) c", c=3))
                nc.vector.tensor_tensor(out=r3[:, 2:3], in0=r3[:, 0:1], in1=r3[:, 0:1],
                                        op=mybir.AluOpType.mult)
                rhs3.append(r3)
            g3_ps = p1ps.tile([32, 3], F32, tag="g3", name="g3", bufs=1)
            for cc in range(NCH):
                nc.tensor.matmul(g3_ps, selp_sb[:, cc, :], rhs3[cc],
                                 start=(cc == 0), stop=(cc == NCH - 1))
            keepalive(3, dep=rhs3[3])
            g3 = p1sb.tile([32, 3], F32, tag="g3sb", name="g3sb")
            nc.vector.tensor_copy(g3, g3_ps)
            # var_g = mean(var_c) + mean(mean_c^2) - mean_g^2, then rstd via
            # sqrt + reciprocal + one (fused) Newton-Raphson step
            ve = p1sb.tile([32, 1], F32, tag="ve", name="ve")
            nc.vector.tensor_tensor(out=ve, in0=g3[:, 1:2], in1=g3[:, 2:3],
                                    op=mybir.AluOpType.add)
            mg2 = p1sb.tile([32, 1], F32, tag="mg2", name="mg2")
            nc.vector.tensor_tensor(out=mg2, in0=g3[:, 0:1], in1=g3[:, 0:1],
                                    op=mybir.AluOpType.mult)
            nc.vector.scalar_tensor_tensor(
                out=ve, in0=ve, scalar=EPS, in1=mg2,
                op0=mybir.AluOpType.add, op1=mybir.AluOpType.subtract)
            sd = p1sb.tile([32, 1], F32, tag="sd", name="sd")
            nc.scalar.activation(sd, ve, mybir.ActivationFunctionType.Sqrt)
            keepalive(2, dep=sd)
            y0 = p1sb.tile([32, 1], F32, tag="y0", name="y0")
            nc.vector.reciprocal(y0, sd)
            t1 = p1sb.tile([32, 1], F32, tag="t1", name="t1")
            nc.vector.scalar_tensor_tensor(out=t1, in0=ve, scalar=y0, in1=y0,
                                           op0=mybir.AluOpType.mult,
                                           op1=mybir.AluOpType.mult)
            nc.vector.tensor_scalar(out=t1, in0=t1, scalar1=-0.5, scalar2=1.5,
                                    op0=mybir.AluOpType.mult, op1=mybir.AluOpType.add)
            g2 = p1sb.tile([32, 2], F32, tag="g2sb", name="g2sb")
            nc.vector.tensor_copy(g2[:, 0:1], g3[:, 0:1])
            nc.vector.tensor_tensor(out=g2[:, 1:2], in0=y0, in1=t1,
                                    op=mybir.AluOpType.mult)
            keepalive(2, dep=g2)
            # broadcast group (mean, rstd) to per-channel scale/bias in one
            # vectorized pass: bcall[:, 2cc:2cc+2] = (mean, rstd) of chunk cc
            bps = p1ps.tile([128, 2 * NCH], F32, tag="bps", name="bps", bufs=1)
            for cc in range(NCH):
                nc.tensor.matmul(bps[:, 2 * cc:2 * cc + 2],
                                 sel_sb[:, cc * 128:(cc + 1) * 128], g2,
                                 start=True, stop=True)
            bcall = p1sb.tile([128, NCH, 2], F32, tag="bcall", name="bcall")
            nc.vector.tensor_copy(bcall, bps)
            gam_r = gamma_sb.rearrange("p (a b) -> p a b", b=1)
            bet_r = beta_sb.rearrange("p (a b) -> p a b", b=1)
            nc.vector.tensor_tensor(out=scale_all, in0=bcall[:, :, 1:2],
                                    in1=gam_r, op=mybir.AluOpType.mult)
            mtall = p1sb.tile([128, NCH, 1], F32, tag="mtall", name="mtall")
            nc.vector.tensor_tensor(out=mtall, in0=bcall[:, :, 0:1],
                                    in1=scale_all, op=mybir.AluOpType.mult)
            nc.vector.tensor_tensor(out=bias_all, in0=bet_r, in1=mtall,
                                    op=mybir.AluOpType.subtract)

        # ====== P2: normalize h (resident) -> V and qk = (wq wk^T) h_q,
        # ====== then P3: attention - one pool scope, no barrier between them
        with tc.tile_pool(name="p3ps", bufs=1, space="PSUM") as p3ps, \
             tc.tile_pool(name="p3ot", bufs=1, space="PSUM") as p3ot, \
             tc.tile_pool(name="p3sb", bufs=1) as p3sb, \
             tc.tile_pool(name="p3pt", bufs=32) as p3pt:
            for w in range(NW):
                wsl = slice(w * 512, (w + 1) * 512)
                for p in range(NPAIR):
                    for j in range(2):
                        cc = 2 * p + j
                        if j == 0:
                            nc.vector.tensor_scalar(
                                out=hw_win[w][p][:, j, :], in0=xt_t[p][:, j, wsl],
                                scalar1=scale_t[cc], scalar2=bias_t[cc],
                                op0=mybir.AluOpType.mult, op1=mybir.AluOpType.add)
                        else:
                            nc.scalar.activation(
                                hw_win[w][p][:, j, :], xt_t[p][:, j, wsl],
                                mybir.ActivationFunctionType.Identity,
                                bias=bias_t[cc], scale=scale_t[cc])
                for i in range(4):
                    ps = p3ps.tile([128, 512], F32, tag="sc", name="kvp", bufs=3)
                    for p in range(NPAIR):
                        nc.tensor.matmul(
                            ps, hw_win[w][p][:, :, i * 128:(i + 1) * 128],
                            wv_t[p], start=(p == 0), stop=(p == NPAIR - 1),
                            perf_mode=DRM)
                    if i < 2:
                        nc.vector.tensor_copy(v_big[:, w * 4 + i, :], ps)
                    else:
                        nc.scalar.copy(v_big[:, w * 4 + i, :], ps)
                if w < NQW:
                    for cq in range(NCH):
                        ps = p3ps.tile([128, 512], F32, tag="sc", name="kvp", bufs=3)
                        for p in range(NPAIR):
                            nc.tensor.matmul(
                                ps, wkq_t[p][:, :, cq * 128:(cq + 1) * 128],
                                hw_win[w][p],
                                start=(p == 0), stop=(p == NPAIR - 1), perf_mode=DRM)
                        if cq < 2:
                            nc.vector.tensor_scalar(
                                out=qts_t[cq // 2][:, cq % 2, w * 512:(w + 1) * 512],
                                in0=ps, scalar1=bq_sb[:, cq:cq + 1], scalar2=None,
                                op0=mybir.AluOpType.add)
                        else:
                            nc.scalar.activation(
                                qts_t[cq // 2][:, cq % 2, w * 512:(w + 1) * 512], ps,
                                mybir.ActivationFunctionType.Identity,
                                bias=bq_sb[:, cq:cq + 1])
            # ---- P3: attention, scores and exp@V fused per key-subtile ----
            # Per m: score matmuls for m, then PV matmuls for m-1 (whose exp
            # just finished on ACT) and the m-1 rowsum - the PE never waits
            # for the scalar engine, and the softmax-denominator reciprocal
            # chain is emitted only after all PE work so it overlaps PV.
            NM = NKT // 2

            def emit_proj(blk, ots):
                # output projection + residual for a finished block
                for sub in range(NSUB):
                    ti = blk * NSUB + sub
                    ps_p = p3ps.tile([128, C], F32, tag="sc", name="ps_p", bufs=3)
                    for p in range(NPAIR):
                        nc.tensor.matmul(
                            ps_p, ots[p][:, :, sub * 128:(sub + 1) * 128], wp_t[p],
                            start=(p == 0), stop=(p == NPAIR - 1), perf_mode=DRM)
                    xres = p3sb.tile([128, C], F32, tag="xres", name="xres", bufs=3)
                    nc.sync.dma_start(out=xres, in_=xresb_h[ti * 128:(ti + 1) * 128, :])
                    fin = p3sb.tile([128, C], F32, tag="fin", name="fin", bufs=3)
                    nc.vector.tensor_tensor(out=fin, in0=ps_p, in1=xres,
                                            op=mybir.AluOpType.add)
                    nc.sync.dma_start(out=out_h[ti * 128:(ti + 1) * 128, :], in_=fin)

            pending = []
            for blk in range(NBLK):
                q0 = blk * 512
                ptws = []
                rs_ps = p3ot.tile([1, 512], F32, tag="rsum", name="rsum", bufs=1)
                ot_ps = p3ot.tile([128, NCH, 512], F32, tag="ot", name="ot", bufs=1)

                def pv_step(m, rs_ps=rs_ps, ot_ps=ot_ps, ptws=ptws):
                    nc.tensor.matmul(rs_ps, ones8[:, :, 0:1], ptws[m],
                                     start=(m == 0), stop=(m == NM - 1),
                                     perf_mode=DRM)
                    for cv in range(NCH):
                        nc.tensor.matmul(
                            ot_ps[:, cv, :],
                            v_big[:, 2 * m:2 * m + 2, cv * 128:(cv + 1) * 128],
                            ptws[m], start=(m == 0), stop=(m == NM - 1),
                            perf_mode=DRM)

                for m in range(NM):
                    ptw = p3pt.tile([128, 2, 512], F8, tag="ptw", name="ptw")
                    for h in range(2):
                        w2 = 2 * m + h
                        st_ps = p3ps.tile([128, 512], F32, tag="sc", name="st_ps", bufs=3)
                        for p in range(NPAIR):
                            nc.tensor.matmul(
                                st_ps,
                                hw_win[w2 // 4][p][:, :, (w2 % 4) * 128:(w2 % 4 + 1) * 128],
                                qts_t[p][:, :, q0:q0 + 512],
                                start=(p == 0), stop=(p == NPAIR - 1), perf_mode=DRM)
                        nc.scalar.activation(ptw[:, h, :], st_ps,
                                             mybir.ActivationFunctionType.Exp,
                                             bias=shift_t, scale=SCALE)
                    ptws.append(ptw)
                    if m > 0:
                        pv_step(m - 1)
                    if m == 6 and pending:
                        # previous block's projection, emitted mid-stream so
                        # its PSUM-evac dependencies are long satisfied
                        emit_proj(*pending.pop())
                pv_step(NM - 1)
                rs_row = p3sb.tile([1, 512], F32, tag="rs_row", name="rs_row", bufs=2)
                nc.scalar.copy(rs_row, rs_ps)
                rsb = p3sb.tile([128, 512], F32, tag="rsb", name="rsb", bufs=2)
                nc.gpsimd.partition_broadcast(rsb, rs_row[0:1, :])
                rinvb = p3sb.tile([128, 512], F32, tag="rinvb", name="rinvb", bufs=2)
                nc.vector.reciprocal_approx_fast(rinvb, rsb)
                # normalize rows (deferred softmax denominator) -> fp8
                ots = [p3sb.tile([128, 2, 512], F8, tag=f"ots{pp}", name=f"ots{pp}",
                                 bufs=2) for pp in range(NPAIR)]
                for cv in range(NCH):
                    nc.vector.tensor_tensor(out=ots[cv // 2][:, cv % 2, :],
                                            in0=ot_ps[:, cv, :], in1=rinvb,
                                            op=mybir.AluOpType.mult)
                pending.append((blk, ots))
            emit_proj(*pending.pop())

    nc.compile()
    return nc


_NC_CACHE = []


def prepare_in_maps(x, gamma, beta, wq, bq, wk, bk, wv, bv, wp, bp):
    import ml_dtypes
    F8NP = ml_dtypes.float8_e4m3

    def to8(a):
        return np.ascontiguousarray(
            np.clip(np.asarray(a, np.float32), -240.0, 240.0).astype(F8NP))

    def pair_interleave(wm):
        # [C, N] -> [NPAIR, 128, 2, N]; element [p, ci, j, n] = wm[(2p+j)*128+ci, n]
        wm = np.asarray(wm, np.float32)
        return to8(wm.reshape(2, 2, 128, -1).transpose(0, 2, 1, 3))

    x = np.ascontiguousarray(np.asarray(x, dtype=np.float32))
    xf = x.reshape(B, T, C)
    bpp = (np.asarray(bv, np.float32) @ np.asarray(wp, np.float32)
           + np.asarray(bp, np.float32))
    sel = np.zeros((32, 512), np.float32)
    selpool = np.zeros((128, 4, 32), np.float32)
    for cc in range(4):
        for cl in range(128):
            sel[8 * cc + cl // GSIZE, cc * 128 + cl] = 1.0
            selpool[cl, cc, 8 * cc + cl // GSIZE] = 1.0 / GSIZE
    wkqt = np.asarray(wq, np.float32) @ np.asarray(wk, np.float32).T
    common = {
        "wkq": pair_interleave(wkqt),
        "wv": pair_interleave(wv), "wp": pair_interleave(wp),
        "bq": np.asarray(wk, np.float32) @ np.asarray(bq, np.float32),
        "gamma": np.asarray(gamma, np.float32),
        "beta": np.asarray(beta, np.float32),
        "selmat": sel,
        "selpool": selpool,
        "ones8": np.ones((128, 2, 16), F8NP),
    }
    in_maps = []
    for core in range(NCORES):
        b, qoff = core // 4, (core % 4) * QS
        # rotate so this core's query strip is rows 0..1023 (attention and
        # group stats are permutation-invariant over tokens)
        xr = np.roll(xf[b], -qoff, axis=0)           # [T, C]
        xtp = pair_interleave(xr.T)                  # [NPAIR, 128, 2, T]
        in_maps.append({
            **common,
            "xt": xtp,
            "xresb": np.ascontiguousarray(xf[b, qoff:qoff + QS] + bpp[None, :]),
        })
    return in_maps


def kernel(x, gamma, beta, wq, bq, wk, bk, wv, bv, wp, bp):
    if not _NC_CACHE:
        _NC_CACHE.append(_build())
    nc = _NC_CACHE[0]
    in_maps = prepare_in_maps(x, gamma, beta, wq, bq, wk, bk, wv, bv, wp, bp)
    res = run_bass_kernel_spmd(nc, in_maps, list(range(NCORES)))
    out = np.empty((B, T, C), np.float32)
    for core in range(NCORES):
        b, qoff = core // 4, (core % 4) * QS
        out[b, qoff:qoff + QS] = res.results[core]["out"]
    return out.reshape(B, H, W, C)
